# revision 8
# baseline (speedup 1.0000x reference)
"""MoE feed-forward (8 experts, top-2) on 8 Trainium2 NeuronCores.

Expert-parallel: core e holds expert e's weights. The (cheap) router runs on
host; tokens are dispatched to expert cores host-side, each core runs
  y = cw * (gelu(x @ W1 + b1) @ W2 + b2)
for its tokens in float32r (TF32-like, full PE rate), and the host combines
the per-expert contributions back into token order.

Fixed problem shape (hardcoded per contest contract):
  x [4, 2048, 1024], Wr [8, 1024], W1 [8, 1024, 4096], b1 [8, 4096],
  W2 [8, 4096, 1024], b2 [8, 1024]. TOP_K = 2.
"""

import math

import numpy as np

import concourse.bass as bass
import concourse.mybir as mybir
import concourse.tile as tile
from concourse import bacc
from concourse.bass import ts
from concourse.bass_utils import run_bass_kernel_spmd

D = 1024  # d_model
F = 4096  # ff dim
E = 8  # experts == cores
TOP_K = 2
CAP = 2304  # tokens per expert-core (mean load 2048; overflow -> host)
# token blocks (moving free dim); 512 keeps the per-matmul f32r weight
# load (~193ns, not elidable) fully hidden under the 512-row compute
BLOCKS = [(0, 512), (512, 512), (1024, 512), (1536, 512), (2048, 256)]
BLK = 512
KD = D // 128  # 8   contraction tiles for GEMM1
KF = F // 128  # 32  contraction tiles for GEMM2
MF = F // 128  # 32  output tiles for GEMM1
MD = D // 128  # 8   output tiles for GEMM2

F32 = mybir.dt.float32
F32R = mybir.dt.float32r

# Cache the built+finalized Bass graph across kernel() calls in one process.
_NC_CACHE = {}

LAST_RESULT = None  # BassKernelResults of the most recent device run


def _build_nc():
    nc = bacc.Bacc("TRN2", target_bir_lowering=False)

    xT = nc.declare_dram_parameter("xT", [128, KD, CAP], F32R, isOutput=False)
    w1 = nc.declare_dram_parameter("w1", [128, KD, F], F32R, isOutput=False)
    b1c = nc.declare_dram_parameter("b1c", [128, MF], F32, isOutput=False)
    w2 = nc.declare_dram_parameter("w2", [128, KF, D], F32R, isOutput=False)
    b2c = nc.declare_dram_parameter("b2c", [128, MD], F32, isOutput=False)
    cwb = nc.declare_dram_parameter("cwb", [128, CAP], F32, isOutput=False)
    yT = nc.declare_dram_parameter("yT", [128, MD, CAP], F32, isOutput=True)

    hT = nc.dram_tensor("hT", [128, MF, CAP], F32R)

    with tile.TileContext(nc) as tc:
        with (
            tc.tile_pool(name="w", bufs=1) as wpool,
            tc.tile_pool(name="blkio", bufs=4) as bpool,
            tc.tile_pool(name="stage", bufs=2) as spool,
            tc.tile_pool(name="const", bufs=1) as cpool,
            tc.tile_pool(name="ps", bufs=4, space="PSUM") as pspool,
        ):
            b1t = cpool.tile([128, MF], F32)
            nc.sync.dma_start(out=b1t[:], in_=b1c[:])
            b2t = cpool.tile([128, MD], F32)
            nc.sync.dma_start(out=b2t[:], in_=b2c[:])

            # ---------------- Phase 1: hT = gelu(W1.T @ xT + b1) ----------------
            w1t = wpool.tile([128, KD, F], F32R, tag="w")
            nc.sync.dma_start(out=w1t[:], in_=w1[:])

            for off, bw in BLOCKS:
                xb = bpool.tile([128, KD, BLK], F32R, tag="xb")
                nc.sync.dma_start(out=xb[:, :, :bw], in_=xT[:, :, off : off + bw])
                for m in range(MF):
                    ps = pspool.tile([128, BLK], F32, tag="ps")
                    for k in range(KD):
                        nc.tensor.matmul(
                            ps[:, :bw],
                            lhsT=w1t[:, k, ts(m, 128)],
                            rhs=xb[:, k, :bw],
                            start=(k == 0),
                            stop=(k == KD - 1),
                        )
                    hs = spool.tile([128, BLK], F32R, tag="hs")
                    nc.scalar.activation(
                        hs[:, :bw],
                        ps[:, :bw],
                        mybir.ActivationFunctionType.Gelu,
                        bias=b1t[:, m : m + 1],
                    )
                    nc.sync.dma_start(out=hT[:, m, off : off + bw], in_=hs[:, :bw])

            # ------------- Phase 2: yT = cw * (W2.T @ hT + b2) -------------
            w2t = wpool.tile([128, KF, D], F32R, tag="w")  # reuses w1t's slot
            nc.sync.dma_start(out=w2t[:], in_=w2[:])

            for off, bw in BLOCKS:
                cb = spool.tile([128, BLK], F32, tag="cb")
                nc.sync.dma_start(out=cb[:, :bw], in_=cwb[:, off : off + bw])
                hbs = []
                for kc in range(4):
                    hb = bpool.tile([128, KD, BLK], F32R, tag="xb")
                    nc.sync.dma_start(
                        out=hb[:, :, :bw], in_=hT[:, ts(kc, KD), off : off + bw]
                    )
                    hbs.append(hb)
                for m in range(MD):
                    ps = pspool.tile([128, BLK], F32, tag="ps")
                    for kc in range(4):
                        for k in range(KD):
                            nc.tensor.matmul(
                                ps[:, :bw],
                                lhsT=w2t[:, kc * KD + k, ts(m, 128)],
                                rhs=hbs[kc][:, k, :bw],
                                start=(kc == 0 and k == 0),
                                stop=(kc == 3 and k == KD - 1),
                            )
                    ys = spool.tile([128, BLK], F32, tag="ys")
                    nc.scalar.activation(
                        ys[:, :bw],
                        ps[:, :bw],
                        mybir.ActivationFunctionType.Identity,
                        bias=b2t[:, m : m + 1],
                    )
                    nc.vector.tensor_mul(
                        out=ys[:, :bw], in0=ys[:, :bw], in1=cb[:, :bw]
                    )
                    nc.sync.dma_start(out=yT[:, m, off : off + bw], in_=ys[:, :bw])

    nc.finalize()
    return nc


def _gelu_exact_np(x):
    try:
        from scipy.special import erf

        return 0.5 * x * (1.0 + erf(x / np.sqrt(2.0)))
    except ImportError:
        _erf = np.vectorize(math.erf)
        return 0.5 * x * (1.0 + _erf(x / np.sqrt(2.0)))


def _route(t, Wr):
    """Replicate the reference router in fp32 numpy: softmax + top-2 with
    jax.lax.top_k tie-breaking (first index wins), weights renormalized."""
    logits = t @ Wr.T  # [T, E] fp32
    mx = logits.max(axis=1, keepdims=True)
    ez = np.exp(logits - mx, dtype=np.float32)
    probs = ez / ez.sum(axis=1, keepdims=True, dtype=np.float32)

    arange = np.arange(t.shape[0])
    i1 = probs.argmax(axis=1)
    masked = probs.copy()
    masked[arange, i1] = -np.inf
    i2 = masked.argmax(axis=1)
    v1 = probs[arange, i1]
    v2 = probs[arange, i2]
    s = v1 + v2
    return i1, i2, v1 / s, v2 / s


def kernel(x, Wr, W1, b1, W2, b2):
    global LAST_RESULT

    x = np.asarray(x, dtype=np.float32)
    Wr = np.asarray(Wr, dtype=np.float32)
    W1 = np.asarray(W1, dtype=np.float32)
    b1 = np.asarray(b1, dtype=np.float32)
    W2 = np.asarray(W2, dtype=np.float32)
    b2 = np.asarray(b2, dtype=np.float32)

    Bb, Ss, _ = x.shape
    T = Bb * Ss
    t = np.ascontiguousarray(x.reshape(T, D))

    i1, i2, cw1, cw2 = _route(t, Wr)

    # per-expert token lists (device handles first CAP; remainder -> host)
    dev_idx, dev_cw, host_idx, host_cw = [], [], [], []
    for e in range(E):
        sel1 = np.nonzero(i1 == e)[0]
        sel2 = np.nonzero(i2 == e)[0]
        idx = np.concatenate([sel1, sel2])
        cw = np.concatenate([cw1[sel1], cw2[sel2]]).astype(np.float32)
        dev_idx.append(idx[:CAP])
        dev_cw.append(cw[:CAP])
        host_idx.append(idx[CAP:])
        host_cw.append(cw[CAP:])

    in_maps = []
    for e in range(E):
        idx = dev_idx[e]
        n = len(idx)
        xe = np.zeros((128, KD, CAP), dtype=np.float32)
        # t[idx] : [n, D] -> [n, KD, 128] -> [128, KD, n]
        xe[:, :, :n] = t[idx].reshape(n, KD, 128).transpose(2, 1, 0)
        cwe = np.zeros((CAP,), dtype=np.float32)
        cwe[:n] = dev_cw[e]
        in_maps.append(
            {
                "xT": xe,
                "w1": np.ascontiguousarray(
                    W1[e].reshape(KD, 128, F).transpose(1, 0, 2)
                ),
                "b1c": np.ascontiguousarray(b1[e].reshape(MF, 128).T),
                "w2": np.ascontiguousarray(
                    W2[e].reshape(KF, 128, D).transpose(1, 0, 2)
                ),
                "b2c": np.ascontiguousarray(b2[e].reshape(MD, 128).T),
                "cwb": np.broadcast_to(cwe, (128, CAP)).copy(),
            }
        )

    if "nc" not in _NC_CACHE:
        _NC_CACHE["nc"] = _build_nc()
    nc = _NC_CACHE["nc"]

    res = run_bass_kernel_spmd(nc, in_maps, core_ids=list(range(E)))
    LAST_RESULT = res

    out = np.zeros((T, D), dtype=np.float32)
    for e in range(E):
        idx = dev_idx[e]
        n = len(idx)
        if n == 0:
            continue
        yT = res.results[e]["yT"]  # [128, MD, CAP]
        ye = yT.transpose(2, 1, 0).reshape(CAP, D)[:n]  # [n, D]
        out[idx] += ye

    # exact host fallback for (rare) capacity overflow
    for e in range(E):
        idx = host_idx[e]
        if len(idx) == 0:
            continue
        h = _gelu_exact_np(t[idx] @ W1[e] + b1[e]).astype(np.float32)
        ye = (h @ W2[e] + b2[e]) * host_cw[e][:, None]
        out[idx] += ye.astype(np.float32)

    return out.reshape(Bb, Ss, D)


# revision 9
# speedup vs baseline: 1.2342x; 1.2342x over previous
"""MoE feed-forward (8 experts, top-2) on 8 Trainium2 NeuronCores.

Expert-parallel: core e holds expert e's weights. The (cheap) router runs on
host; tokens are dispatched to expert cores host-side, each core runs
  y = cw * (gelu(x @ W1 + b1) @ W2 + b2)
for its tokens, and the host combines the per-expert contributions back
into token order.

Device numerics: matmul operands in fp16 (PSUM accumulation fp32, GELU/bias/
combine-weights fp32) -> ~6e-4 relative error overall.

Fixed problem shape (hardcoded per contest contract):
  x [4, 2048, 1024], Wr [8, 1024], W1 [8, 1024, 4096], b1 [8, 4096],
  W2 [8, 4096, 1024], b2 [8, 1024]. TOP_K = 2.
"""

import math

import numpy as np

import concourse.bass as bass
import concourse.mybir as mybir
import concourse.tile as tile
from concourse import bacc
from concourse.bass import ts
from concourse.bass_utils import run_bass_kernel_spmd

D = 1024  # d_model
F = 4096  # ff dim
E = 8  # experts == cores
TOP_K = 2
CAP = 2304  # tokens per expert-core (mean load 2048; overflow -> host)
# token blocks (moving free dim); 512 keeps the per-matmul weight load hidden
BLOCKS = [(0, 512), (512, 512), (1024, 512), (1536, 512), (2048, 256)]
BLK = 512
KD = D // 128  # 8   contraction tiles for GEMM1
KF = F // 128  # 32  contraction tiles for GEMM2
MF = F // 128  # 32  output tiles for GEMM1
MD = D // 128  # 8   output tiles for GEMM2

F32 = mybir.dt.float32
F16 = mybir.dt.float16

# Cache the built+finalized Bass graph across kernel() calls in one process.
_NC_CACHE = {}

LAST_RESULT = None  # BassKernelResults of the most recent device run


def _build_nc():
    nc = bacc.Bacc("TRN2", target_bir_lowering=False)

    xT = nc.declare_dram_parameter("xT", [128, KD, CAP], F16, isOutput=False)
    # weights pre-chunked by output tile on host so each m-slice is contiguous
    w1 = nc.declare_dram_parameter("w1", [MF, 128, KD, 128], F16, isOutput=False)
    b1c = nc.declare_dram_parameter("b1c", [128, MF], F32, isOutput=False)
    w2 = nc.declare_dram_parameter("w2", [MD, 128, KF, 128], F16, isOutput=False)
    b2c = nc.declare_dram_parameter("b2c", [128, MD], F32, isOutput=False)
    cwb = nc.declare_dram_parameter("cwb", [128, CAP], F32, isOutput=False)
    yT = nc.declare_dram_parameter("yT", [128, MD, CAP], F32, isOutput=True)

    hT = nc.dram_tensor("hT", [128, MF, CAP], F16)

    with tile.TileContext(nc) as tc:
        with (
            tc.tile_pool(name="w", bufs=1) as wpool,
            tc.tile_pool(name="blkio", bufs=8) as bpool,
            tc.tile_pool(name="stage", bufs=3) as spool,
            tc.tile_pool(name="const", bufs=1) as cpool,
            tc.tile_pool(name="ps", bufs=4, space="PSUM") as pspool,
        ):
            b1t = cpool.tile([128, MF], F32)
            nc.sync.dma_start(out=b1t[:], in_=b1c[:])
            b2t = cpool.tile([128, MD], F32)
            nc.sync.dma_start(out=b2t[:], in_=b2c[:])

            # ---------------- Phase 1: hT = gelu(W1.T @ xT + b1) ----------------
            # m-sliced weight loads: first matmul only waits for slice 0
            w1t = wpool.tile([128, KD, F], F16, tag="w")
            for m in range(MF):
                nc.sync.dma_start(
                    out=w1t[:, :, ts(m, 128)],
                    in_=w1[m].rearrange("p k q -> p k q"),
                )

            for off, bw in BLOCKS:
                xb = bpool.tile([128, KD, BLK], F16, tag="xb")
                nc.sync.dma_start(out=xb[:, :, :bw], in_=xT[:, :, off : off + bw])
                for m in range(MF):
                    ps = pspool.tile([128, BLK], F32, tag="ps")
                    for k in range(KD):
                        nc.tensor.matmul(
                            ps[:, :bw],
                            lhsT=w1t[:, k, ts(m, 128)],
                            rhs=xb[:, k, :bw],
                            start=(k == 0),
                            stop=(k == KD - 1),
                        )
                    hs = spool.tile([128, BLK], F16, tag="hs")
                    nc.scalar.activation(
                        hs[:, :bw],
                        ps[:, :bw],
                        mybir.ActivationFunctionType.Gelu,
                        bias=b1t[:, m : m + 1],
                    )
                    nc.sync.dma_start(out=hT[:, m, off : off + bw], in_=hs[:, :bw])

            # ------------- Phase 2: yT = cw * (W2.T @ hT + b2) -------------
            w2t = wpool.tile([128, KF, D], F16, tag="w")  # reuses w1t's slot
            for m in range(MD):
                nc.sync.dma_start(
                    out=w2t[:, :, ts(m, 128)],
                    in_=w2[m].rearrange("p k q -> p k q"),
                )

            for off, bw in BLOCKS:
                cb = spool.tile([128, BLK], F32, tag="cb")
                nc.sync.dma_start(out=cb[:, :bw], in_=cwb[:, off : off + bw])
                hbs = []
                for kc in range(4):
                    hb = bpool.tile([128, KD, BLK], F16, tag="xb")
                    nc.sync.dma_start(
                        out=hb[:, :, :bw], in_=hT[:, ts(kc, KD), off : off + bw]
                    )
                    hbs.append(hb)
                for m in range(MD):
                    ps = pspool.tile([128, BLK], F32, tag="ps")
                    for kc in range(4):
                        for k in range(KD):
                            nc.tensor.matmul(
                                ps[:, :bw],
                                lhsT=w2t[:, kc * KD + k, ts(m, 128)],
                                rhs=hbs[kc][:, k, :bw],
                                start=(kc == 0 and k == 0),
                                stop=(kc == 3 and k == KD - 1),
                            )
                    ys = spool.tile([128, BLK], F32, tag="ys")
                    nc.scalar.activation(
                        ys[:, :bw],
                        ps[:, :bw],
                        mybir.ActivationFunctionType.Identity,
                        bias=b2t[:, m : m + 1],
                    )
                    nc.vector.tensor_mul(
                        out=ys[:, :bw], in0=ys[:, :bw], in1=cb[:, :bw]
                    )
                    nc.sync.dma_start(out=yT[:, m, off : off + bw], in_=ys[:, :bw])

    nc.finalize()
    return nc


def _gelu_exact_np(x):
    try:
        from scipy.special import erf

        return 0.5 * x * (1.0 + erf(x / np.sqrt(2.0)))
    except ImportError:
        _erf = np.vectorize(math.erf)
        return 0.5 * x * (1.0 + _erf(x / np.sqrt(2.0)))


def _route(t, Wr):
    """Replicate the reference router in fp32 numpy: softmax + top-2 with
    jax.lax.top_k tie-breaking (first index wins), weights renormalized."""
    logits = t @ Wr.T  # [T, E] fp32
    mx = logits.max(axis=1, keepdims=True)
    ez = np.exp(logits - mx, dtype=np.float32)
    probs = ez / ez.sum(axis=1, keepdims=True, dtype=np.float32)

    arange = np.arange(t.shape[0])
    i1 = probs.argmax(axis=1)
    masked = probs.copy()
    masked[arange, i1] = -np.inf
    i2 = masked.argmax(axis=1)
    v1 = probs[arange, i1]
    v2 = probs[arange, i2]
    s = v1 + v2
    return i1, i2, v1 / s, v2 / s


def kernel(x, Wr, W1, b1, W2, b2):
    global LAST_RESULT

    x = np.asarray(x, dtype=np.float32)
    Wr = np.asarray(Wr, dtype=np.float32)
    W1 = np.asarray(W1, dtype=np.float32)
    b1 = np.asarray(b1, dtype=np.float32)
    W2 = np.asarray(W2, dtype=np.float32)
    b2 = np.asarray(b2, dtype=np.float32)

    Bb, Ss, _ = x.shape
    T = Bb * Ss
    t = np.ascontiguousarray(x.reshape(T, D))

    i1, i2, cw1, cw2 = _route(t, Wr)

    # per-expert token lists (device handles first CAP; remainder -> host)
    dev_idx, dev_cw, host_idx, host_cw = [], [], [], []
    for e in range(E):
        sel1 = np.nonzero(i1 == e)[0]
        sel2 = np.nonzero(i2 == e)[0]
        idx = np.concatenate([sel1, sel2])
        cw = np.concatenate([cw1[sel1], cw2[sel2]]).astype(np.float32)
        dev_idx.append(idx[:CAP])
        dev_cw.append(cw[:CAP])
        host_idx.append(idx[CAP:])
        host_cw.append(cw[CAP:])

    in_maps = []
    for e in range(E):
        idx = dev_idx[e]
        n = len(idx)
        xe = np.zeros((128, KD, CAP), dtype=np.float16)
        # t[idx] : [n, D] -> [n, KD, 128] -> [128, KD, n]
        xe[:, :, :n] = t[idx].reshape(n, KD, 128).transpose(2, 1, 0)
        cwe = np.zeros((CAP,), dtype=np.float32)
        cwe[:n] = dev_cw[e]
        # W1[e]: [D, F] -> [m][p][k][q] with row index k*128+p, col index m*128+q
        w1e = np.ascontiguousarray(
            W1[e].reshape(KD, 128, MF, 128).transpose(2, 1, 0, 3), dtype=np.float16
        )
        w2e = np.ascontiguousarray(
            W2[e].reshape(KF, 128, MD, 128).transpose(2, 1, 0, 3), dtype=np.float16
        )
        in_maps.append(
            {
                "xT": xe,
                "w1": w1e,
                "b1c": np.ascontiguousarray(b1[e].reshape(MF, 128).T),
                "w2": w2e,
                "b2c": np.ascontiguousarray(b2[e].reshape(MD, 128).T),
                "cwb": np.broadcast_to(cwe, (128, CAP)).copy(),
            }
        )

    if "nc" not in _NC_CACHE:
        _NC_CACHE["nc"] = _build_nc()
    nc = _NC_CACHE["nc"]

    res = run_bass_kernel_spmd(nc, in_maps, core_ids=list(range(E)))
    LAST_RESULT = res

    out = np.zeros((T, D), dtype=np.float32)
    for e in range(E):
        idx = dev_idx[e]
        n = len(idx)
        if n == 0:
            continue
        yT = res.results[e]["yT"]  # [128, MD, CAP]
        ye = yT.transpose(2, 1, 0).reshape(CAP, D)[:n]  # [n, D]
        out[idx] += ye

    # exact host fallback for (rare) capacity overflow
    for e in range(E):
        idx = host_idx[e]
        if len(idx) == 0:
            continue
        h = _gelu_exact_np(t[idx] @ W1[e] + b1[e]).astype(np.float32)
        ye = (h @ W2[e] + b2[e]) * host_cw[e][:, None]
        out[idx] += ye.astype(np.float32)

    return out.reshape(Bb, Ss, D)
